# revision 17
# baseline (speedup 1.0000x reference)
"""Trainium2 Bass kernel for nn_DCLMBlock (B=4, S=2048, H=1536) on 8 NeuronCores.

Sharding: token-parallel. Core c handles batch b=c//2, sequence half h=c%2
(1024 tokens). Every core receives a causal *context* of 2048 tokens ending at
its last own token (zero-padded in front for first halves), so one SPMD
program serves all cores.

v2 (this file): fp8-e4m3 DoubleRow matmuls for the state-path gate, the
mix-gate, the mixing matmul, the FFN-in gate half and half of the FFN-out
contraction (error-budgeted via a host-side numpy emulator: predicted
rel-err ~0.016 < 2e-2). Depthwise-conv tap chains are split across the
Vector engine (tensor_scalar 4x + tensor_tensor 2x form) and the otherwise
idle GpSimd engine (fused scalar_tensor_tensor form); head-conv windows
shrink per layer to the receptive field actually needed. Everything else as
v1: feature-major activations, fp32 PSUM, tanh-form sigmoids, host-side
fp32 residual sum of the three path outputs.
"""

import numpy as np
import ml_dtypes

import concourse.bass as bass
import concourse.tile as tile
from concourse import mybir
from concourse.bass_utils import run_bass_kernel_spmd

F32 = mybir.dt.float32
BF16 = mybir.dt.bfloat16
F8 = mybir.dt.float8e4
AF = mybir.ActivationFunctionType
OP = mybir.AluOpType
DR = mybir.MatmulPerfMode.DoubleRow
BF = ml_dtypes.bfloat16
F8NP = ml_dtypes.float8_e4m3

B, S, H = 4, 2048, 1536
NH, HD = 12, 128
KK = 4
INNER = 6144
EPS = 1e-6
CONV_DILS = (1, 2, 4, 8, 16, 32)
HEAD_DILS = [(1, 2, 4), (1, 1, 1), (4, 8, 16), (8, 16, 32), (32, 64, 128),
             (64, 128, 256), (256, 512, 1024), (1, 100, 200), (1, 500, 1000),
             (1, 1024, 2048), (3, 9, 27), (5, 25, 125)]

HEAD_HALO = [3 * sum(ds) for ds in HEAD_DILS]
NKH = H // 128          # 12 feature tiles
NKI = INNER // 128      # 48 inner tiles
N_OUT8 = 24             # ffn_out inner k-tiles done in fp8 (of 48)
CTX = 2048              # context tokens per core
OWN = 1024              # own tokens per core
W = 1216                # conv-stack window (192 halo + 1024 own)
HEAD_WIN = [min(CTX, OWN + h) for h in HEAD_HALO]
WPAD = 192
N_CORES = 8


def _chunks_for(win):
    """Token chunks [pos, pos+ln) covering [CTX-win, CTX), cut on the
    512 grid so each chunk fits one PSUM tile and never straddles the
    ctx/own tile boundary at 1024."""
    pos = CTX - win
    out = []
    while pos < CTX:
        ln = min((512 - pos % 512) if pos % 512 else 512, CTX - pos)
        out.append((pos, ln))
        pos += ln
    return out


# ---------------------------------------------------------------------------
# walrus sync-wait capacity workaround (this build allows <=1 wait per inst)
# ---------------------------------------------------------------------------
def _fix_sync_capacity(nc, dummy_ap):
    ET = mybir.EngineType
    was_frozen = nc._frozen
    nc._frozen = False

    def mk_stub(engine_ty, waits, updates):
        if engine_ty == ET.SP:
            inst = nc.sync.nop(nofuse=True)
        elif engine_ty == ET.DVE:
            inst = nc.vector.tensor_copy(dummy_ap, dummy_ap)
        elif engine_ty == ET.Activation:
            inst = nc.scalar.activation(dummy_ap, dummy_ap, AF.Copy)
        elif engine_ty == ET.PE:
            inst = nc.tensor.drain()
        elif engine_ty == ET.Pool:
            inst = nc.gpsimd.tensor_copy(dummy_ap, dummy_ap)
        else:
            raise RuntimeError(f"no stub for engine {engine_ty}")
        popped = nc.cur_bb.bb.instructions.pop()
        assert popped is inst.ins
        inst.ins.sync_info = mybir.SyncInfo(
            on_wait=list(waits), on_update=list(updates)
        )
        return inst.ins

    for f in nc.m.functions:
        for bb in f.blocks:
            out = []
            changed = False
            for ins in bb.instructions:
                si = ins.sync_info
                if si is not None and len(si.on_wait) > 1:
                    for wt in si.on_wait[1:]:
                        out.append(mk_stub(ins.engine, [wt], []))
                    del si.on_wait[1:]
                    changed = True
                out.append(ins)
                if si is not None and len(si.on_update) > 1:
                    for u in si.on_update[1:]:
                        out.append(mk_stub(ins.engine, [], [u]))
                    del si.on_update[1:]
                    changed = True
            if changed:
                bb.instructions[:] = out
    nc._frozen = was_frozen


# ---------------------------------------------------------------------------
# device program
# ---------------------------------------------------------------------------
def _build():
    nc = bass.Bass()
    dummy = nc.sbuf_tensor([1, 1], F32).__enter__()

    def par(name, shape, dt):
        return nc.declare_dram_parameter(name, shape, dt, isOutput=False)

    xc_d = par("xc", [NKH, 128, CTX], BF16)
    cw_d = par("cw", [128, NKH, 6, KK], F32)
    cb_d = par("cb", [128, NKH, 6], F32)
    nw1_d = par("nw1", [128, NKH], F32)
    hww_d = par("hww", [128, NH, 3, KK], F32)
    cprof_d = par("cprof", [NH, 128, OWN], BF16)
    mask_d = par("mask", [128, W], BF16)
    oneh_d = par("oneh", [NH, NH * 128], BF16)
    wg8_d = par("wg8", [24, 128, NKH, 128], F8)
    wr_d = par("wr", [128, NKH, NH], BF16)
    rb_d = par("rb", [NH, 1], F32)
    wpg_d = par("wpg", [128, NKH, 3], BF16)
    pgb_d = par("pgb", [3, 1], F32)
    wproj_d = par("wproj", [NKH, 128, NKH, 128], BF16)
    projb_d = par("projb", [128, NKH], F32)
    wmg8_d = par("wmg8", [NKH, 128, NKH, 128], F8)
    mgb_d = par("mgb", [128, NKH], F32)   # pre-halved on host (tanh sigmoid)
    wmix8_d = par("wmix8", [NKH, 128, NKH, 128], F8)  # pre-halved on host
    mixb_d = par("mixb", [128, NKH], F32)
    wfic_d = par("wfic", [NKI, 128, NKH, 128], BF16)
    wfig8_d = par("wfig8", [NKI, 128, NKH, 128], F8)
    wfo8_d = par("wfo8", [2, N_OUT8 // 2, 128, 2, 6, 128], F8)
    wfo16_d = par("wfo16", [2, NKI - N_OUT8, 128, 6, 128], BF16)

    po_c = nc.declare_dram_parameter("po_c", [NKH, 2, 128, 512], BF16, isOutput=True)
    po_s = nc.declare_dram_parameter("po_s", [NKH, 2, 128, 512], BF16, isOutput=True)
    po_f = nc.declare_dram_parameter("po_f", [NKH, 2, 128, 512], BF16, isOutput=True)

    dma = nc.sync.dma_start

    from contextlib import ExitStack
    with tile.TileContext(nc) as tc:
        with ExitStack() as es:
            ec = es.enter_context
            const = ec(tc.tile_pool(name="const", bufs=1))
            psp = ec(tc.tile_pool(name="ps", bufs=8, space="PSUM"))
            xno = ec(tc.tile_pool(name="xno", bufs=NKH))   # xn own halves
            xn8op = ec(tc.tile_pool(name="xn8o", bufs=NKH // 2))
            w8sl = ec(tc.tile_pool(name="w8sl", bufs=2))
            smalls = ec(tc.tile_pool(name="smalls", bufs=1))
            g3bp = ec(tc.tile_pool(name="g3b", bufs=1))
            sgp = ec(tc.tile_pool(name="sgt", bufs=2))
            ftp = ec(tc.tile_pool(name="ftmp", bufs=1))

            # =========== Phase R: load x first, rmsnorm via Rsqrt ===========
            esR = ExitStack()
            xcp = esR.enter_context(tc.tile_pool(name="xc", bufs=NKH))
            x2p = esR.enter_context(tc.tile_pool(name="x2", bufs=3))
            rtp = esR.enter_context(tc.tile_pool(name="rt", bufs=1))
            rbcp = esR.enter_context(tc.tile_pool(name="rbc", bufs=1))
            hstp = tc.alloc_tile_pool(name="hst", bufs=NKH, side="right")
            caccp = tc.alloc_tile_pool(name="cacc", bufs=2, side="right")
            cgelp = tc.alloc_tile_pool(name="cgel", bufs=2, side="right")
            xn8cp = tc.alloc_tile_pool(name="xn8c", bufs=NKH // 2,
                                       side="right")
            xnc = tc.alloc_tile_pool(name="xnc", bufs=NKH, side="right")

            xct = []
            for t in range(NKH):
                xt = xcp.tile([128, CTX], BF16, tag="xc")
                dma(xt[:], xc_d[t])
                xct.append(xt)

            # ---- constants (issued after x so x lands first) ----
            t_cw = const.tile([128, NKH, 6, KK], F32)
            dma(t_cw[:], cw_d[:])
            t_cb = const.tile([128, NKH, 6], F32)
            dma(t_cb[:], cb_d[:])
            t_nw1 = const.tile([128, NKH], F32)
            dma(t_nw1[:], nw1_d[:])
            t_hww = const.tile([128, NH, 3, KK], F32)
            dma(t_hww[:], hww_d[:])
            t_oneh = const.tile([NH, NH * 128], BF16)
            dma(t_oneh[:], oneh_d[:])
            t_wr = const.tile([128, NKH, NH], BF16)
            dma(t_wr[:], wr_d[:])
            t_rb = const.tile([NH, 1], F32)
            dma(t_rb[:], rb_d[:])
            t_wpg = const.tile([128, NKH, 3], BF16)
            dma(t_wpg[:], wpg_d[:])
            t_pgb = const.tile([3, 1], F32)
            dma(t_pgb[:], pgb_d[:])
            t_projb = const.tile([128, NKH], F32)
            dma(t_projb[:], projb_d[:])
            t_mgb = const.tile([128, NKH], F32)
            dma(t_mgb[:], mgb_d[:])
            t_mixb = const.tile([128, NKH], F32)
            dma(t_mixb[:], mixb_d[:])
            ones_k = const.tile([128, 1], BF16)
            nc.vector.memset(ones_k[:], 1.0)
            eps_t = const.tile([1, 1], F32)
            nc.vector.memset(eps_t[:], EPS)

            # sum of squares -> sqrt -> 1/x -> bf16 row -> broadcast
            ones_b = rtp.tile([1, 128], BF16, tag="ones_b")
            nc.vector.memset(ones_b[:], 1.0)
            sqf = rtp.tile([1, CTX], F32, tag="sqf")
            r1 = rtp.tile([1, CTX], BF16, tag="r1")
            for c in range(4):
                cs = slice(512 * c, 512 * (c + 1))
                pss = psp.tile([1, 512], F32, tag="ps")
                for t in range(NKH):
                    x2t = x2p.tile([128, 512], BF16, tag="x2")
                    nc.vector.tensor_mul(x2t[:], xct[t][:, cs], xct[t][:, cs])
                    nc.tensor.matmul(pss[:], ones_k[:], x2t[:],
                                     start=(t == 0), stop=(t == NKH - 1))
                nc.scalar.activation(sqf[:, cs], pss[:], AF.Sqrt,
                                     bias=eps_t[:], scale=1.0 / H)
            rcf = rtp.tile([1, CTX], F32, tag="rcf")
            for c in range(4):
                cs = slice(512 * c, 512 * (c + 1))
                nc.vector.reciprocal(rcf[:, cs], sqf[:, cs])
                nc.vector.tensor_copy(r1[:, cs], rcf[:, cs])
            rb128 = rbcp.tile([128, CTX], BF16)
            for c in range(4):
                cs = slice(512 * c, 512 * (c + 1))
                pb = psp.tile([128, 512], F32, tag="ps")
                nc.tensor.matmul(pb[:], ones_b[:], r1[:, cs],
                                 start=True, stop=True)
                nc.scalar.activation(rb128[:, cs], pb[:], AF.Copy)

            # fp8 pair tiles first: the gate matmuls depend only on these
            xn8c = []
            xn8o = []
            for q in range(NKH // 2):
                t8 = xn8cp.tile([128, 2, OWN], F8, tag="xn8c")
                xn8c.append(t8)
                t8 = xn8op.tile([128, 2, OWN], F8, tag="xn8o")
                xn8o.append(t8)
            for t in range(NKH):
                nc.vector.tensor_mul(xn8c[t // 2][:, t % 2, :],
                                     xct[t][:, :OWN], rb128[:, :OWN])
                nc.vector.tensor_mul(xn8o[t // 2][:, t % 2, :],
                                     xct[t][:, OWN:], rb128[:, OWN:])
            # xn bf16: own full, ctx only the conv-stack tail window
            xn_o = []
            for t in range(NKH):
                xt = xno.tile([128, OWN], BF16, tag="xno")
                nc.vector.tensor_mul(xt[:], xct[t][:, OWN:], rb128[:, OWN:])
                xn_o.append(xt)
            xn_c = []
            for t in range(NKH):
                xt = xnc.tile([128, W - OWN], BF16, tag="xnc")
                nc.vector.tensor_mul(xt[:], xct[t][:, CTX - W:OWN],
                                     rb128[:, CTX - W:OWN])
                xn_c.append(xt)

            def xn8_sl(q, pos, ln):
                """fp8 pair slice at token window [pos, pos+ln)."""
                if pos < OWN:
                    return xn8c[q][:, :, pos:pos + ln]
                return xn8o[q][:, :, pos - OWN:pos - OWN + ln]

            esR.close()

            maskb = smalls.tile([128, W], BF16, tag="maskb")
            dma(maskb[:], mask_d[:])
            hs = []
            for t in range(NKH):
                ht = hstp.tile([128, W], BF16, tag="h")
                # window covers ctx tokens [832:1024) and own [1024:2048)
                nc.vector.tensor_scalar_mul(ht[:, :W - OWN],
                                            xn_c[t][:],
                                            t_nw1[:, t:t + 1])
                nc.vector.tensor_scalar_mul(ht[:, W - OWN:],
                                            xn_o[t][:],
                                            t_nw1[:, t:t + 1])
                hs.append(ht)
            xnc.release()
            def stack_unit(t, j, gp):
                d = CONV_DILS[j]
                acc = caccp.tile([128, W], BF16, tag="cacc")
                nc.vector.tensor_scalar_mul(acc[:], maskb[:],
                                            t_cb[:, t, j:j + 1])
                for k in (3, 2, 1, 0):
                    delta = (3 - k) * d
                    if gp:
                        # tap multiply on the Act engine, add on DVE
                        tmp = cgelp.tile([128, W], BF16, tag="cgel")
                        nc.scalar.activation(tmp[:, :W - delta],
                                             hs[t][:, :W - delta], AF.Copy,
                                             scale=t_cw[:, t, j, k:k + 1])
                        nc.vector.tensor_add(acc[:, delta:], acc[:, delta:],
                                             tmp[:, :W - delta])
                    else:
                        nc.vector.scalar_tensor_tensor(
                            acc[:, delta:], hs[t][:, :W - delta],
                            t_cw[:, t, j, k:k + 1], acc[:, delta:],
                            OP.mult, OP.add)
                ge = cgelp.tile([128, W], BF16, tag="cgel")
                nc.scalar.activation(ge[:], acc[:], AF.Gelu)
                nc.gpsimd.tensor_add(hs[t][:], hs[t][:], ge[:])

            stack_q = [(t, j) for j in range(6) for t in range(NKH)]
            stack_i = [0]
            stk_ctr = [0]
            def emit_stack():
                if stack_i[0] >= len(stack_q):
                    return False
                t, j = stack_q[stack_i[0]]
                stack_i[0] += 1
                stk_ctr[0] += 1
                stack_unit(t, j, gp=(stk_ctr[0] % 2 == 0))
                return True

            # =========== Phase G: gate matmul (fp8 DoubleRow) + x_gated ====
            xgES = ExitStack()
            xgp = xgES.enter_context(tc.tile_pool(name="xg", bufs=1))
            xg8p_pool = tc.alloc_tile_pool(name="xg8", bufs=NH // 2)
            xg = []
            for t in range(NH):
                wc = w8sl.tile([128, NKH, 128], F8, tag="w8sl")
                dma(wc[:], wg8_d[t])
                wgt = w8sl.tile([128, NKH, 128], F8, tag="w8sl")
                dma(wgt[:], wg8_d[t + NH])
                ws0 = CTX - HEAD_WIN[t]
                xgt = xgp.tile([128, HEAD_WIN[t]], BF16, tag=f"xg{t}")
                for pos, ln in _chunks_for(HEAD_WIN[t]):
                    pc = psp.tile([128, 512], F32, tag="ps")
                    for q in range(NKH // 2):
                        nc.tensor.matmul(pc[:, :ln], wc[:, 2 * q:2 * q + 2, :],
                                         xn8_sl(q, pos, ln),
                                         start=(q == 0), stop=(q == NKH // 2 - 1),
                                         perf_mode=DR)
                    pg = psp.tile([128, 512], F32, tag="ps")
                    for q in range(NKH // 2):
                        nc.tensor.matmul(pg[:, :ln], wgt[:, 2 * q:2 * q + 2, :],
                                         xn8_sl(q, pos, ln),
                                         start=(q == 0), stop=(q == NKH // 2 - 1),
                                         perf_mode=DR)
                    # x_gated = content * sigmoid(gate)
                    #         = (0.5*content) * (1 + tanh(gate/2))
                    pcb = sgp.tile([128, 512], BF16, tag="pcb")
                    nc.scalar.activation(pcb[:, :ln], pc[:, :ln], AF.Copy,
                                         scale=0.5)
                    sg = sgp.tile([128, 512], BF16, tag="sg")
                    nc.scalar.activation(sg[:, :ln], pg[:, :ln], AF.Tanh,
                                         scale=0.5)
                    nc.vector.scalar_tensor_tensor(
                        xgt[:, pos - ws0:pos - ws0 + ln], sg[:, :ln],
                        1.0, pcb[:, :ln], OP.add, OP.mult)
                xg.append(xgt)
                emit_stack()
                emit_stack()
            xn8cp.release()

            # =========== Phase P: path gates, router, mask broadcast =========
            pgs = smalls.tile([3, OWN], BF16, tag="pgs")
            for c in range(2):
                cs_own = slice(512 * c, 512 * (c + 1))
                pp = psp.tile([3, 512], F32, tag="ps")
                for kt in range(NKH):
                    nc.tensor.matmul(pp[:], t_wpg[:, kt, :],
                                     xn_o[kt][:, cs_own],
                                     start=(kt == 0), stop=(kt == NKH - 1))
                nc.scalar.activation(pgs[:, cs_own], pp[:],
                                     AF.Sigmoid, bias=t_pgb[:, 0:1])
            def pg_bcast(pool, i, tag):
                gt = pool.tile([128, OWN], BF16, tag=tag, name=f"g3b_{i}")
                for c in range(2):
                    cs = slice(512 * c, 512 * (c + 1))
                    pb = psp.tile([128, 512], F32, tag="ps")
                    nc.tensor.matmul(pb[:], t_oneh[0:3, 128 * i:128 * (i + 1)],
                                     pgs[:, cs], start=True, stop=True)
                    nc.scalar.activation(gt[:, cs], pb[:], AF.Copy)
                return gt
            gfb = pg_bcast(g3bp, 2, "g3b")
            hw_sig = smalls.tile([NH, OWN], BF16, tag="hw_sig")
            for c in range(2):
                cs_own = slice(512 * c, 512 * (c + 1))
                pr = psp.tile([NH, 512], F32, tag="ps")
                for kt in range(NKH):
                    nc.tensor.matmul(pr[:], t_wr[:, kt, :],
                                     xn_o[kt][:, cs_own],
                                     start=(kt == 0), stop=(kt == NKH - 1))
                nc.scalar.activation(hw_sig[:, cs_own], pr[:],
                                     AF.Sigmoid, bias=t_rb[:, 0:1])


            xg8 = [xg8p_pool.tile([128, 2, OWN], F8, tag="xg8", name=f"xg8_{q}")
                   for q in range(NH // 2)]

            # DVE work units: conv-stack convs, head convs, head outputs.
            # gp=True routes the tap chain to the GpSimd engine (fused stt);
            # gp=False uses the DVE tensor_scalar(4x)+tensor_tensor(2x) form.
            def head_unit(i, j, gp):
                d = HEAD_DILS[i][j]
                wh = HEAD_WIN[i]
                nj = min(wh, OWN + 3 * sum(HEAD_DILS[i][j + 1:]))
                oj = wh - nj
                acc = haccp.tile([128, CTX], BF16, tag="hacc")
                nc.vector.tensor_scalar_mul(acc[:, oj:wh], xg[i][:, oj:wh],
                                               t_hww[:, i, j, 3:4])
                for k in (2, 1, 0):
                    delta = (3 - k) * d
                    a = max(oj, delta)
                    if a >= wh:
                        continue
                    nc.vector.scalar_tensor_tensor(
                        acc[:, a:wh], xg[i][:, a - delta:wh - delta],
                        t_hww[:, i, j, k:k + 1], acc[:, a:wh],
                        OP.mult, OP.add)
                nc.gpsimd.tensor_add(xg[i][:, oj:wh], xg[i][:, oj:wh],
                                       acc[:, oj:wh])

            def headout_unit(i):
                off = HEAD_WIN[i] - OWN
                cp = cpfp.tile([128, OWN], BF16, tag="cpf")
                dma(cp[:], cprof_d[i])
                nc.gpsimd.tensor_add(xg[i][:, off:], xg[i][:, off:], cp[:])
                for c in range(2):
                    cs = slice(512 * c, 512 * (c + 1))
                    pb = psp.tile([128, 512], F32, tag="ps")
                    nc.tensor.matmul(pb[:], t_oneh[:, 128 * i:128 * (i + 1)],
                                     hw_sig[:, cs], start=True, stop=True)
                    hb = sgp.tile([128, 512], BF16, tag="sg")
                    nc.scalar.activation(hb[:], pb[:], AF.Copy)
                    cso = slice(off + 512 * c, off + 512 * (c + 1))
                    nc.gpsimd.tensor_mul(xg[i][:, cso], xg[i][:, cso], hb[:])
                    # fp8 copy for the mix-gate matmul
                    nc.scalar.activation(xg8[i // 2][:, i % 2, cs],
                                         xg[i][:, cso], AF.Copy)

            head_q = [(i, j) for i in range(NH) for j in range(4)]
            head_i = [0]

            def emit_conv_unit():
                if head_i[0] < len(head_q):
                    i, j = head_q[head_i[0]]
                    head_i[0] += 1
                    if j == 3:
                        headout_unit(i)
                    else:
                        stk_ctr[0] += 1
                        head_unit(i, j, gp=(stk_ctr[0] % 2 == 0))
                    return True
                return emit_stack()

            # =========== Phase F: GLU FFN (own tokens), hff in SBUF =========
            ffnES = ExitStack()
            hffp = ffnES.enter_context(tc.tile_pool(name="hff", bufs=NKI - N_OUT8))
            hff8p = ffnES.enter_context(tc.tile_pool(name="hff8", bufs=N_OUT8 // 2))
            wfip = ffnES.enter_context(tc.tile_pool(name="wfi", bufs=2))
            wfi8p = ffnES.enter_context(tc.tile_pool(name="wfi8", bufs=2))
            wfo8pp = ffnES.enter_context(tc.tile_pool(name="wfo8p", bufs=2))
            wfopp = ffnES.enter_context(tc.tile_pool(name="wfop", bufs=2))
            headES = ExitStack()
            haccp = headES.enter_context(tc.tile_pool(name="hacc", bufs=2))
            cpfp = headES.enter_context(tc.tile_pool(name="cpf", bufs=1))

            for c in range(2):
                cs_own = slice(512 * c, 512 * (c + 1))
                hffc = []      # bf16 tiles for inner k-tiles N_OUT8..47
                hff8c = []     # fp8 pair tiles for inner k-tiles 0..N_OUT8-1
                for q in range(N_OUT8 // 2):
                    hff8c.append(hff8p.tile([128, 2, 512], F8, tag="hff8", name=f"hff8_{c}_{q}"))
                for p in range(NKI):
                    wcs = wfip.tile([128, NKH, 128], BF16, tag="wfi")
                    dma(wcs[:], wfic_d[p])
                    wgs = wfi8p.tile([128, NKH, 128], F8, tag="wfi8")
                    dma(wgs[:], wfig8_d[p])
                    pc = psp.tile([128, 512], F32, tag="ps")
                    for kt in range(NKH):
                        nc.tensor.matmul(pc[:], wcs[:, kt, :],
                                         xn_o[kt][:, cs_own],
                                         start=(kt == 0), stop=(kt == NKH - 1))
                    pg = psp.tile([128, 512], F32, tag="ps")
                    for q in range(NKH // 2):
                        nc.tensor.matmul(pg[:], wgs[:, 2 * q:2 * q + 2, :],
                                         xn8o[q][:, :, cs_own],
                                         start=(q == 0), stop=(q == NKH // 2 - 1),
                                         perf_mode=DR)
                    pcb = sgp.tile([128, 512], BF16, tag="pcb")
                    nc.scalar.activation(pcb[:], pc[:], AF.Copy, scale=0.5)
                    sg = sgp.tile([128, 512], BF16, tag="sg")
                    nc.scalar.activation(sg[:], pg[:], AF.Tanh, scale=0.5)
                    if p < N_OUT8:
                        nc.vector.scalar_tensor_tensor(
                            hff8c[p // 2][:, p % 2, :], sg[:], 1.0, pcb[:],
                            OP.add, OP.mult)
                    else:
                        hoc = hffp.tile([128, 512], BF16, tag="hff")
                        nc.vector.scalar_tensor_tensor(hoc[:], sg[:], 1.0,
                                                       pcb[:], OP.add, OP.mult)
                        hffc.append(hoc)
                    emit_conv_unit()
                    if p % 4 == 0:
                        emit_conv_unit()

                # ffn_out: 2 groups of 6 output tiles, hff from SBUF
                for g in range(2):
                    pss = []
                    for _j in range(6):
                        psj = psp.tile([128, 512], F32, tag="ps")
                        pss.append(psj)
                    for q in range(N_OUT8 // 2):
                        w8 = wfo8pp.tile([128, 2, 6, 128], F8, tag="wfo8")
                        dma(w8[:], wfo8_d[g, q])
                        for j in range(6):
                            nc.tensor.matmul(pss[j][:], w8[:, :, j, :],
                                             hff8c[q][:, :, :],
                                             start=(q == 0), stop=False,
                                             perf_mode=DR,
                                             skip_group_check=True)
                    for kt in range(NKI - N_OUT8):
                        ws = wfopp.tile([128, 6, 128], BF16, tag="wfop")
                        dma(ws[:], wfo16_d[g, kt])
                        for j in range(6):
                            nc.tensor.matmul(pss[j][:], ws[:, j, :],
                                             hffc[kt][:],
                                             start=False,
                                             stop=(kt == NKI - N_OUT8 - 1),
                                             skip_group_check=True)
                    for j in range(6):
                        pfb = sgp.tile([128, 512], BF16, tag="pcb")
                        nc.scalar.activation(pfb[:], pss[j][:], AF.Copy)
                        tf = ftp.tile([128, 512], BF16, tag="ftmp")
                        nc.gpsimd.tensor_mul(tf[:], pfb[:], gfb[:, cs_own])
                        dma(po_f[g * 6 + j, c], tf[:])
            while emit_conv_unit() or emit_stack():
                pass
            headES.close()
            ffnES.close()
            cgelp.release()
            caccp.release()

            # =========== conv-stack projection ===========
            projES = ExitStack()
            wsl = projES.enter_context(tc.tile_pool(name="wsl", bufs=2))
            gcbp = projES.enter_context(tc.tile_pool(name="gcb", bufs=1))
            gcb = pg_bcast(gcbp, 0, "gcb")
            for t in range(NKH):
                wp = wsl.tile([128, NKH, 128], BF16, tag="wsl")
                dma(wp[:], wproj_d[t])
                for c in range(2):
                    ws_ = slice(WPAD + 512 * c, WPAD + 512 * (c + 1))
                    pp = psp.tile([128, 512], F32, tag="ps")
                    for kt in range(NKH):
                        nc.tensor.matmul(pp[:], wp[:, kt, :], hs[kt][:, ws_],
                                         start=(kt == 0), stop=(kt == NKH - 1))
                    tb = sgp.tile([128, 512], BF16, tag="sg")
                    nc.scalar.activation(tb[:], pp[:], AF.Identity,
                                         bias=t_projb[:, t:t + 1])
                    tf = ftp.tile([128, 512], BF16, tag="ftmp")
                    nc.gpsimd.tensor_mul(tf[:], tb[:],
                                         gcb[:, 512 * c:512 * (c + 1)])
                    dma(po_c[t, c], tf[:])
            projES.close()
            hstp.release()

            # =========== Phase M: mix gate + mixing (fp8 DoubleRow) =========
            # sigmoid(z) = 0.5*(1+tanh(z/2)): sgm_t = tanh(0.5*pm + mgb_half),
            # xg2 = (1+sgm_t)*xg  [the 0.5 is folded into wmix on the host]
            sgmES = ExitStack()
            sgmp = sgmES.enter_context(tc.tile_pool(name="sgm", bufs=NH))
            gsbp = sgmES.enter_context(tc.tile_pool(name="gsb", bufs=1))
            gsb = pg_bcast(gsbp, 1, "gsb")
            xg82p_pool = sgmES.enter_context(tc.tile_pool(name="xg82", bufs=NH // 2))
            sgm = []
            for t in range(NKH):
                wm = w8sl.tile([128, NKH, 128], F8, tag="w8sl")
                dma(wm[:], wmg8_d[t])
                st = sgmp.tile([128, OWN], BF16, tag="sgm")
                for c in range(2):
                    pm = psp.tile([128, 512], F32, tag="ps")
                    for q in range(NKH // 2):
                        nc.tensor.matmul(pm[:], wm[:, 2 * q:2 * q + 2, :],
                                         xg8[q][:, :, 512 * c:512 * (c + 1)],
                                         start=(q == 0), stop=(q == NKH // 2 - 1),
                                         perf_mode=DR)
                    nc.scalar.activation(st[:, 512 * c:512 * (c + 1)], pm[:],
                                         AF.Tanh, bias=t_mgb[:, t:t + 1],
                                         scale=0.5)
                sgm.append(st)
            xg82 = [xg82p_pool.tile([128, 2, OWN], F8, tag="xg82", name=f"xg82_{q}")
                    for q in range(NH // 2)]
            for t in range(NKH):
                ot_ = HEAD_WIN[t] - OWN
                nc.vector.scalar_tensor_tensor(
                    xg82[t // 2][:, t % 2, :], sgm[t][:], 1.0,
                    xg[t][:, ot_:], OP.add, OP.mult)
            for t in range(NKH):
                wx = w8sl.tile([128, NKH, 128], F8, tag="w8sl")
                dma(wx[:], wmix8_d[t])
                for c in range(2):
                    pm = psp.tile([128, 512], F32, tag="ps")
                    for q in range(NKH // 2):
                        nc.tensor.matmul(pm[:], wx[:, 2 * q:2 * q + 2, :],
                                         xg82[q][:, :, 512 * c:512 * (c + 1)],
                                         start=(q == 0), stop=(q == NKH // 2 - 1),
                                         perf_mode=DR)
                    tb = sgp.tile([128, 512], BF16, tag="sg")
                    nc.scalar.activation(tb[:], pm[:], AF.Identity,
                                         bias=t_mixb[:, t:t + 1])
                    tf = ftp.tile([128, 512], BF16, tag="ftmp")
                    nc.gpsimd.tensor_mul(tf[:], tb[:],
                                         gsb[:, 512 * c:512 * (c + 1)])
                    dma(po_s[t, c], tf[:])
            sgmES.close()
            xg8p_pool.release()
            xgES.close()

    nc.finalize()
    _fix_sync_capacity(nc, dummy[:])
    return nc


# ---------------------------------------------------------------------------
# host side
# ---------------------------------------------------------------------------
def _wslab(Wt, nk, no):
    """[IN, OUT] weight (already transposed to in-major) -> [no, 128, nk, 128]
    slab layout: slab[ot][p, kt, m] = Wt[kt*128+p, ot*128+m]."""
    return np.ascontiguousarray(
        Wt.reshape(nk, 128, no, 128).transpose(2, 1, 0, 3))


def _head_bias_profile(head_ws, head_bs):
    """Data-independent bias part of each head's (linear) conv chain over the
    global sequence, with exact causal zero padding."""
    C = np.zeros((NH, HD, S), np.float32)
    for i in range(NH):
        v = np.zeros((HD, S), np.float32)
        for j, d in enumerate(HEAD_DILS[i]):
            conv = np.zeros_like(v)
            for k in range(KK):
                delta = (3 - k) * d
                if delta == 0:
                    conv += head_ws[i, j, :, 0, k][:, None] * v
                elif delta < S:
                    conv[:, delta:] += head_ws[i, j, :, 0, k][:, None] * v[:, :-delta]
            v = v + conv + head_bs[i, j][:, None]
        C[i] = v
    return C


_NC_CACHE = {}


def kernel(**inputs):
    x = np.asarray(inputs["x"], np.float32)
    nw = np.asarray(inputs["norm_w"], np.float32)
    conv_ws = np.asarray(inputs["conv_ws"], np.float32)
    conv_bs = np.asarray(inputs["conv_bs"], np.float32)
    conv_proj_w = np.asarray(inputs["conv_proj_w"], np.float32)
    conv_proj_b = np.asarray(inputs["conv_proj_b"], np.float32)
    gate_w = np.asarray(inputs["gate_w"], np.float32)
    router_w = np.asarray(inputs["router_w"], np.float32)
    router_b = np.asarray(inputs["router_b"], np.float32)
    head_ws = np.asarray(inputs["head_ws"], np.float32)
    head_bs = np.asarray(inputs["head_bs"], np.float32)
    mix_gate_w = np.asarray(inputs["mix_gate_w"], np.float32)
    mix_gate_b = np.asarray(inputs["mix_gate_b"], np.float32)
    mixing_w = np.asarray(inputs["mixing_w"], np.float32)
    mixing_b = np.asarray(inputs["mixing_b"], np.float32)
    ffn_in_w = np.asarray(inputs["ffn_in_w"], np.float32)
    ffn_out_w = np.asarray(inputs["ffn_out_w"], np.float32)
    pg_w = np.asarray(inputs["pg_w"], np.float32)
    pg_b = np.asarray(inputs["pg_b"], np.float32)

    wfi_slab = _wslab((ffn_in_w * nw[2][None, :]).T, NKH, 96)  # [96,128,12,128]
    wfo_t = ffn_out_w.T.reshape(NKI, 128, 2, 6, 128)           # kt-major
    # fp8 half: inner k-tiles 0..N_OUT8-1 as pairs [2, P, 128, 2, 6, 128]
    wfo8 = wfo_t[:N_OUT8].reshape(N_OUT8 // 2, 2, 128, 2, 6, 128) \
        .transpose(3, 0, 2, 1, 4, 5)
    wfo16 = wfo_t[N_OUT8:].transpose(2, 0, 1, 3, 4)

    shared = {
        "cw": np.ascontiguousarray(
            conv_ws[:, :, 0, :].reshape(6, NKH, 128, KK).transpose(2, 1, 0, 3)),
        "cb": np.ascontiguousarray(
            conv_bs.reshape(6, NKH, 128).transpose(2, 1, 0)),
        "nw1": np.ascontiguousarray(nw[0].reshape(NKH, 128).T),
        "hww": np.ascontiguousarray(
            head_ws[:, :, :, 0, :].transpose(2, 0, 1, 3)),
        "wg8": _wslab((gate_w * nw[1][None, :]).T, NKH, 24).astype(F8NP),
        "wr": np.ascontiguousarray(
            (router_w * nw[1][None, :]).T.reshape(NKH, 128, NH)
            .transpose(1, 0, 2)).astype(BF),
        "rb": router_b[:, None].astype(np.float32),
        "wpg": np.ascontiguousarray(
            (pg_w * nw).T.reshape(NKH, 128, 3).transpose(1, 0, 2)).astype(BF),
        "pgb": pg_b[:, None].astype(np.float32),
        "wproj": _wslab(conv_proj_w.T, NKH, NKH).astype(BF),
        "projb": np.ascontiguousarray(conv_proj_b.reshape(NKH, 128).T),
        "wmg8": _wslab(mix_gate_w.T, NKH, NKH).astype(F8NP),
        # tanh-form sigmoid: bias pre-halved; 0.5 gate factor folded into wmix
        "mgb": np.ascontiguousarray(mix_gate_b.reshape(NKH, 128).T) * 0.5,
        "wmix8": _wslab(mixing_w.T * 0.5, NKH, NKH).astype(F8NP),
        "mixb": np.ascontiguousarray(mixing_b.reshape(NKH, 128).T),
        "wfic": np.ascontiguousarray(wfi_slab[:NKI]).astype(BF),
        "wfig8": np.ascontiguousarray(wfi_slab[NKI:]).astype(F8NP),
        "wfo8": np.ascontiguousarray(wfo8).astype(F8NP),
        "wfo16": np.ascontiguousarray(wfo16).astype(BF),
    }
    oneh = np.zeros((NH, NH * 128), np.float32)
    for i in range(NH):
        oneh[i, 128 * i:128 * (i + 1)] = 1.0
    shared["oneh"] = oneh.astype(BF)

    cprof = _head_bias_profile(head_ws, head_bs)  # [NH, HD, S]
    cprof_h = [
        np.ascontiguousarray(cprof[:, :, h * OWN:(h + 1) * OWN]).astype(BF)
        for h in range(2)
    ]
    mask_h = []
    m0 = np.zeros((128, W), np.float32)
    m0[:, WPAD:] = 1.0
    mask_h.append(m0.astype(BF))
    mask_h.append(np.ones((128, W), BF))

    in_maps = []
    for core in range(N_CORES):
        b, h = core // 2, core % 2
        if h == 0:
            ctx = np.concatenate(
                [np.zeros((OWN, H), np.float32), x[b, :OWN]], axis=0)
        else:
            ctx = x[b]
        xc = np.ascontiguousarray(ctx.T.reshape(NKH, 128, CTX)).astype(BF)
        m = dict(shared)
        m["xc"] = xc
        m["cprof"] = cprof_h[h]
        m["mask"] = mask_h[h]
        in_maps.append(m)

    key = "nc"
    if key not in _NC_CACHE:
        _NC_CACHE[key] = _build()
    nc = _NC_CACHE[key]

    import os
    trace = bool(os.environ.get("BASS_KERNEL_TRACE"))
    r = run_bass_kernel_spmd(nc, in_maps, list(range(N_CORES)), trace=trace)
    global LAST_EXEC_NS
    LAST_EXEC_NS = r.exec_time_ns
    res = r.results

    out = np.empty((B, S, H), np.float32)
    for core in range(N_CORES):
        b, h = core // 2, core % 2
        total = np.zeros((H, OWN), np.float32)
        for name in ("po_c", "po_s", "po_f"):
            arr = np.asarray(res[core][name]).astype(np.float32)
            total += arr.transpose(0, 2, 1, 3).reshape(H, OWN)
        rows = slice(h * OWN, (h + 1) * OWN)
        out[b, rows, :] = x[b, rows, :] + total.T
    return out


# revision 18
# speedup vs baseline: 1.0722x; 1.0722x over previous
"""Trainium2 Bass kernel for nn_DCLMBlock (B=4, S=2048, H=1536) on 8 NeuronCores.

Sharding: token-parallel. Core c handles batch b=c//2, sequence half h=c%2
(1024 tokens). Every core receives a causal *context* of 2048 tokens ending at
its last own token (zero-padded in front for first halves), so one SPMD
program serves all cores.

v2 (this file): fp8-e4m3 DoubleRow matmuls for the state-path gate, the
mix-gate, the mixing matmul, the FFN-in gate half and half of the FFN-out
contraction (error-budgeted via a host-side numpy emulator: predicted
rel-err ~0.016 < 2e-2). Depthwise-conv tap chains are split across the
Vector engine (tensor_scalar 4x + tensor_tensor 2x form) and the otherwise
idle GpSimd engine (fused scalar_tensor_tensor form); head-conv windows
shrink per layer to the receptive field actually needed. Everything else as
v1: feature-major activations, fp32 PSUM, tanh-form sigmoids, host-side
fp32 residual sum of the three path outputs.
"""

import numpy as np
import ml_dtypes

import concourse.bass as bass
import concourse.tile as tile
from concourse import mybir
from concourse.bass_utils import run_bass_kernel_spmd

F32 = mybir.dt.float32
BF16 = mybir.dt.bfloat16
F8 = mybir.dt.float8e4
AF = mybir.ActivationFunctionType
OP = mybir.AluOpType
DR = mybir.MatmulPerfMode.DoubleRow
BF = ml_dtypes.bfloat16
F8NP = ml_dtypes.float8_e4m3

B, S, H = 4, 2048, 1536
NH, HD = 12, 128
KK = 4
INNER = 6144
EPS = 1e-6
CONV_DILS = (1, 2, 4, 8, 16, 32)
HEAD_DILS = [(1, 2, 4), (1, 1, 1), (4, 8, 16), (8, 16, 32), (32, 64, 128),
             (64, 128, 256), (256, 512, 1024), (1, 100, 200), (1, 500, 1000),
             (1, 1024, 2048), (3, 9, 27), (5, 25, 125)]

HEAD_HALO = [3 * sum(ds) for ds in HEAD_DILS]
NKH = H // 128          # 12 feature tiles
NKI = INNER // 128      # 48 inner tiles
N_OUT8 = 24             # ffn_out inner k-tiles done in fp8 (of 48)
CTX = 2048              # context tokens per core
OWN = 1024              # own tokens per core
W = 1216                # conv-stack window (192 halo + 1024 own)
HEAD_WIN = [min(CTX, OWN + h) for h in HEAD_HALO]
WPAD = 192
N_CORES = 8


def _chunks_for(win):
    """Token chunks [pos, pos+ln) covering [CTX-win, CTX), cut on the
    512 grid so each chunk fits one PSUM tile and never straddles the
    ctx/own tile boundary at 1024."""
    pos = CTX - win
    out = []
    while pos < CTX:
        ln = min((512 - pos % 512) if pos % 512 else 512, CTX - pos)
        out.append((pos, ln))
        pos += ln
    return out


# ---------------------------------------------------------------------------
# walrus sync-wait capacity workaround (this build allows <=1 wait per inst)
# ---------------------------------------------------------------------------
def _fix_sync_capacity(nc, dummy_ap):
    ET = mybir.EngineType
    was_frozen = nc._frozen
    nc._frozen = False

    def mk_stub(engine_ty, waits, updates):
        if engine_ty == ET.SP:
            inst = nc.sync.nop(nofuse=True)
        elif engine_ty == ET.DVE:
            inst = nc.vector.tensor_copy(dummy_ap, dummy_ap)
        elif engine_ty == ET.Activation:
            inst = nc.scalar.activation(dummy_ap, dummy_ap, AF.Copy)
        elif engine_ty == ET.PE:
            inst = nc.tensor.drain()
        elif engine_ty == ET.Pool:
            inst = nc.gpsimd.tensor_copy(dummy_ap, dummy_ap)
        else:
            raise RuntimeError(f"no stub for engine {engine_ty}")
        popped = nc.cur_bb.bb.instructions.pop()
        assert popped is inst.ins
        inst.ins.sync_info = mybir.SyncInfo(
            on_wait=list(waits), on_update=list(updates)
        )
        return inst.ins

    for f in nc.m.functions:
        for bb in f.blocks:
            out = []
            changed = False
            for ins in bb.instructions:
                si = ins.sync_info
                if si is not None and len(si.on_wait) > 1:
                    for wt in si.on_wait[1:]:
                        out.append(mk_stub(ins.engine, [wt], []))
                    del si.on_wait[1:]
                    changed = True
                out.append(ins)
                if si is not None and len(si.on_update) > 1:
                    for u in si.on_update[1:]:
                        out.append(mk_stub(ins.engine, [], [u]))
                    del si.on_update[1:]
                    changed = True
            if changed:
                bb.instructions[:] = out
    nc._frozen = was_frozen


# ---------------------------------------------------------------------------
# device program
# ---------------------------------------------------------------------------
def _build():
    nc = bass.Bass()
    dummy = nc.sbuf_tensor([1, 1], F32).__enter__()

    def par(name, shape, dt):
        return nc.declare_dram_parameter(name, shape, dt, isOutput=False)

    xc_d = par("xc", [NKH, 128, CTX], BF16)
    cw_d = par("cw", [128, NKH, 6, KK], F32)
    cb_d = par("cb", [128, NKH, 6], F32)
    nw1_d = par("nw1", [128, NKH], F32)
    hww_d = par("hww", [128, NH, 3, KK], F32)
    cprof_d = par("cprof", [NH, 128, OWN], BF16)
    mask_d = par("mask", [128, W], BF16)
    oneh_d = par("oneh", [NH, NH * 128], BF16)
    wg8_d = par("wg8", [24, 128, NKH, 128], F8)
    wr_d = par("wr", [128, NKH, NH], BF16)
    rb_d = par("rb", [NH, 1], F32)
    wpg_d = par("wpg", [128, NKH, 3], BF16)
    pgb_d = par("pgb", [3, 1], F32)
    wproj_d = par("wproj", [NKH, 128, NKH, 128], BF16)
    projb_d = par("projb", [128, NKH], F32)
    wmg8_d = par("wmg8", [NKH, 128, NKH, 128], F8)
    mgb_d = par("mgb", [128, NKH], F32)   # pre-halved on host (tanh sigmoid)
    wmix8_d = par("wmix8", [NKH, 128, NKH, 128], F8)  # pre-halved on host
    mixb_d = par("mixb", [128, NKH], F32)
    wfic_d = par("wfic", [NKI, 128, NKH, 128], BF16)
    wfig8_d = par("wfig8", [NKI, 128, NKH, 128], F8)
    wfo8_d = par("wfo8", [2, N_OUT8 // 2, 128, 2, 6, 128], F8)
    wfo16_d = par("wfo16", [2, NKI - N_OUT8, 128, 6, 128], BF16)

    po_c = nc.declare_dram_parameter("po_c", [NKH, 2, 128, 512], BF16, isOutput=True)
    po_s = nc.declare_dram_parameter("po_s", [NKH, 2, 128, 512], BF16, isOutput=True)
    po_f = nc.declare_dram_parameter("po_f", [NKH, 2, 128, 512], BF16, isOutput=True)

    dma = nc.sync.dma_start

    from contextlib import ExitStack
    with tile.TileContext(nc) as tc:
        with ExitStack() as es:
            ec = es.enter_context
            const = ec(tc.tile_pool(name="const", bufs=1))
            psp = ec(tc.tile_pool(name="ps", bufs=8, space="PSUM"))
            xno = ec(tc.tile_pool(name="xno", bufs=NKH))   # xn own halves
            xn8op = ec(tc.tile_pool(name="xn8o", bufs=NKH // 2))
            w8sl = ec(tc.tile_pool(name="w8sl", bufs=2))
            smalls = ec(tc.tile_pool(name="smalls", bufs=1))
            g3bp = ec(tc.tile_pool(name="g3b", bufs=1))
            sgp = ec(tc.tile_pool(name="sgt", bufs=2))
            ftp = ec(tc.tile_pool(name="ftmp", bufs=1))

            # =========== Phase R: load x first, rmsnorm via Rsqrt ===========
            esR = ExitStack()
            xcp = esR.enter_context(tc.tile_pool(name="xc", bufs=NKH))
            x2p = esR.enter_context(tc.tile_pool(name="x2", bufs=3))
            rtp = esR.enter_context(tc.tile_pool(name="rt", bufs=1))
            rbcp = esR.enter_context(tc.tile_pool(name="rbc", bufs=1))
            hstp = tc.alloc_tile_pool(name="hst", bufs=NKH, side="right")
            caccp = tc.alloc_tile_pool(name="cacc", bufs=2, side="right")
            cgelp = tc.alloc_tile_pool(name="cgel", bufs=2, side="right")
            xn8cp = tc.alloc_tile_pool(name="xn8c", bufs=NKH // 2,
                                       side="right")
            xnc = tc.alloc_tile_pool(name="xnc", bufs=NKH, side="right")

            xct = []
            for t in range(NKH):
                xt = xcp.tile([128, CTX], BF16, tag="xc")
                dma(xt[:], xc_d[t])
                xct.append(xt)

            # ---- constants (issued after x so x lands first) ----
            t_cw = const.tile([128, NKH, 6, KK], F32)
            dma(t_cw[:], cw_d[:])
            t_cb = const.tile([128, NKH, 6], F32)
            dma(t_cb[:], cb_d[:])
            t_nw1 = const.tile([128, NKH], F32)
            dma(t_nw1[:], nw1_d[:])
            t_hww = const.tile([128, NH, 3, KK], F32)
            dma(t_hww[:], hww_d[:])
            t_oneh = const.tile([NH, NH * 128], BF16)
            dma(t_oneh[:], oneh_d[:])
            t_wr = const.tile([128, NKH, NH], BF16)
            dma(t_wr[:], wr_d[:])
            t_rb = const.tile([NH, 1], F32)
            dma(t_rb[:], rb_d[:])
            t_wpg = const.tile([128, NKH, 3], BF16)
            dma(t_wpg[:], wpg_d[:])
            t_pgb = const.tile([3, 1], F32)
            dma(t_pgb[:], pgb_d[:])
            t_projb = const.tile([128, NKH], F32)
            dma(t_projb[:], projb_d[:])
            t_mgb = const.tile([128, NKH], F32)
            dma(t_mgb[:], mgb_d[:])
            t_mixb = const.tile([128, NKH], F32)
            dma(t_mixb[:], mixb_d[:])
            ones_k = const.tile([128, 1], BF16)
            nc.vector.memset(ones_k[:], 1.0)
            eps_t = const.tile([1, 1], F32)
            nc.vector.memset(eps_t[:], EPS)

            # sum of squares -> sqrt -> 1/x -> bf16 row -> broadcast
            ones_b = rtp.tile([1, 128], BF16, tag="ones_b")
            nc.vector.memset(ones_b[:], 1.0)
            sqf = rtp.tile([1, CTX], F32, tag="sqf")
            r1 = rtp.tile([1, CTX], BF16, tag="r1")
            for c in range(4):
                cs = slice(512 * c, 512 * (c + 1))
                pss = psp.tile([1, 512], F32, tag="ps")
                for t in range(NKH):
                    x2t = x2p.tile([128, 512], BF16, tag="x2")
                    nc.vector.tensor_mul(x2t[:], xct[t][:, cs], xct[t][:, cs])
                    nc.tensor.matmul(pss[:], ones_k[:], x2t[:],
                                     start=(t == 0), stop=(t == NKH - 1))
                nc.scalar.activation(sqf[:, cs], pss[:], AF.Sqrt,
                                     bias=eps_t[:], scale=1.0 / H)
            rcf = rtp.tile([1, CTX], F32, tag="rcf")
            for c in range(4):
                cs = slice(512 * c, 512 * (c + 1))
                nc.vector.reciprocal(rcf[:, cs], sqf[:, cs])
                nc.vector.tensor_copy(r1[:, cs], rcf[:, cs])
            rb128 = rbcp.tile([128, CTX], BF16)
            for c in range(4):
                cs = slice(512 * c, 512 * (c + 1))
                pb = psp.tile([128, 512], F32, tag="ps")
                nc.tensor.matmul(pb[:], ones_b[:], r1[:, cs],
                                 start=True, stop=True)
                nc.scalar.activation(rb128[:, cs], pb[:], AF.Copy)

            # fp8 pair tiles first: the gate matmuls depend only on these
            xn8c = []
            xn8o = []
            for q in range(NKH // 2):
                t8 = xn8cp.tile([128, 2, OWN], F8, tag="xn8c")
                xn8c.append(t8)
                t8 = xn8op.tile([128, 2, OWN], F8, tag="xn8o")
                xn8o.append(t8)
            for t in range(NKH):
                nc.vector.tensor_mul(xn8c[t // 2][:, t % 2, :],
                                     xct[t][:, :OWN], rb128[:, :OWN])
                nc.vector.tensor_mul(xn8o[t // 2][:, t % 2, :],
                                     xct[t][:, OWN:], rb128[:, OWN:])
            # xn bf16: own full, ctx only the conv-stack tail window
            xn_o = []
            for t in range(NKH):
                xt = xno.tile([128, OWN], BF16, tag="xno")
                nc.vector.tensor_mul(xt[:], xct[t][:, OWN:], rb128[:, OWN:])
                xn_o.append(xt)
            xn_c = []
            for t in range(NKH):
                xt = xnc.tile([128, W - OWN], BF16, tag="xnc")
                nc.vector.tensor_mul(xt[:], xct[t][:, CTX - W:OWN],
                                     rb128[:, CTX - W:OWN])
                xn_c.append(xt)

            def xn8_sl(q, pos, ln):
                """fp8 pair slice at token window [pos, pos+ln)."""
                if pos < OWN:
                    return xn8c[q][:, :, pos:pos + ln]
                return xn8o[q][:, :, pos - OWN:pos - OWN + ln]

            esR.close()

            maskb = smalls.tile([128, W], BF16, tag="maskb")
            dma(maskb[:], mask_d[:])
            hs = []
            for t in range(NKH):
                ht = hstp.tile([128, W], BF16, tag="h")
                # window covers ctx tokens [832:1024) and own [1024:2048)
                nc.vector.tensor_scalar_mul(ht[:, :W - OWN],
                                            xn_c[t][:],
                                            t_nw1[:, t:t + 1])
                nc.vector.tensor_scalar_mul(ht[:, W - OWN:],
                                            xn_o[t][:],
                                            t_nw1[:, t:t + 1])
                hs.append(ht)
            xnc.release()
            def stack_unit(t, j, gp):
                d = CONV_DILS[j]
                acc = caccp.tile([128, W], BF16, tag="cacc")
                nc.vector.tensor_scalar_mul(acc[:], maskb[:],
                                            t_cb[:, t, j:j + 1])
                for k in (3, 2, 1, 0):
                    delta = (3 - k) * d
                    if gp:
                        # tap multiply on the Act engine, add on DVE
                        tmp = cgelp.tile([128, W], BF16, tag="cgel")
                        nc.scalar.activation(tmp[:, :W - delta],
                                             hs[t][:, :W - delta], AF.Copy,
                                             scale=t_cw[:, t, j, k:k + 1])
                        nc.vector.tensor_add(acc[:, delta:], acc[:, delta:],
                                             tmp[:, :W - delta])
                    else:
                        nc.vector.scalar_tensor_tensor(
                            acc[:, delta:], hs[t][:, :W - delta],
                            t_cw[:, t, j, k:k + 1], acc[:, delta:],
                            OP.mult, OP.add)
                ge = cgelp.tile([128, W], BF16, tag="cgel")
                nc.scalar.activation(ge[:], acc[:], AF.Gelu)
                nc.gpsimd.tensor_add(hs[t][:], hs[t][:], ge[:])

            stack_q = [(t, j) for j in range(6) for t in range(NKH)]
            stack_i = [0]
            stk_ctr = [0]
            def emit_stack():
                if stack_i[0] >= len(stack_q):
                    return False
                t, j = stack_q[stack_i[0]]
                stack_i[0] += 1
                stk_ctr[0] += 1
                stack_unit(t, j, gp=(stk_ctr[0] % 2 == 0))
                return True

            # =========== Phase G: gate matmul (fp8 DoubleRow) + x_gated ====
            xgES = ExitStack()
            xgp = xgES.enter_context(tc.tile_pool(name="xg", bufs=1))
            xg8p_pool = tc.alloc_tile_pool(name="xg8", bufs=NH // 2)
            xg = []
            for t in range(NH):
                wc = w8sl.tile([128, NKH, 128], F8, tag="w8sl")
                dma(wc[:], wg8_d[t])
                wgt = w8sl.tile([128, NKH, 128], F8, tag="w8sl")
                dma(wgt[:], wg8_d[t + NH])
                ws0 = CTX - HEAD_WIN[t]
                xgt = xgp.tile([128, HEAD_WIN[t]], BF16, tag=f"xg{t}")
                for pos, ln in _chunks_for(HEAD_WIN[t]):
                    pc = psp.tile([128, 512], F32, tag="ps")
                    for q in range(NKH // 2):
                        nc.tensor.matmul(pc[:, :ln], wc[:, 2 * q:2 * q + 2, :],
                                         xn8_sl(q, pos, ln),
                                         start=(q == 0), stop=(q == NKH // 2 - 1),
                                         perf_mode=DR)
                    pg = psp.tile([128, 512], F32, tag="ps")
                    for q in range(NKH // 2):
                        nc.tensor.matmul(pg[:, :ln], wgt[:, 2 * q:2 * q + 2, :],
                                         xn8_sl(q, pos, ln),
                                         start=(q == 0), stop=(q == NKH // 2 - 1),
                                         perf_mode=DR)
                    # x_gated = content * sigmoid(gate)
                    #         = (0.5*content) * (1 + tanh(gate/2));
                    # 0.5 folded into the content weights on the host, so the
                    # stt reads the content PSUM directly.
                    sg = sgp.tile([128, 512], BF16, tag="sg")
                    nc.scalar.activation(sg[:, :ln], pg[:, :ln], AF.Tanh,
                                         scale=0.5)
                    nc.vector.scalar_tensor_tensor(
                        xgt[:, pos - ws0:pos - ws0 + ln], sg[:, :ln],
                        1.0, pc[:, :ln], OP.add, OP.mult)
                xg.append(xgt)
            xn8cp.release()

            # =========== Phase P: path gates, router, mask broadcast =========
            pgs = smalls.tile([3, OWN], BF16, tag="pgs")
            for c in range(2):
                cs_own = slice(512 * c, 512 * (c + 1))
                pp = psp.tile([3, 512], F32, tag="ps")
                for kt in range(NKH):
                    nc.tensor.matmul(pp[:], t_wpg[:, kt, :],
                                     xn_o[kt][:, cs_own],
                                     start=(kt == 0), stop=(kt == NKH - 1))
                nc.scalar.activation(pgs[:, cs_own], pp[:],
                                     AF.Sigmoid, bias=t_pgb[:, 0:1])
            def pg_bcast(pool, i, tag):
                gt = pool.tile([128, OWN], BF16, tag=tag, name=f"g3b_{i}")
                for c in range(2):
                    cs = slice(512 * c, 512 * (c + 1))
                    pb = psp.tile([128, 512], F32, tag="ps")
                    nc.tensor.matmul(pb[:], t_oneh[0:3, 128 * i:128 * (i + 1)],
                                     pgs[:, cs], start=True, stop=True)
                    nc.scalar.activation(gt[:, cs], pb[:], AF.Copy)
                return gt
            gfb = pg_bcast(g3bp, 2, "g3b")
            hw_sig = smalls.tile([NH, OWN], BF16, tag="hw_sig")
            for c in range(2):
                cs_own = slice(512 * c, 512 * (c + 1))
                pr = psp.tile([NH, 512], F32, tag="ps")
                for kt in range(NKH):
                    nc.tensor.matmul(pr[:], t_wr[:, kt, :],
                                     xn_o[kt][:, cs_own],
                                     start=(kt == 0), stop=(kt == NKH - 1))
                nc.scalar.activation(hw_sig[:, cs_own], pr[:],
                                     AF.Sigmoid, bias=t_rb[:, 0:1])


            xg8 = [xg8p_pool.tile([128, 2, OWN], F8, tag="xg8", name=f"xg8_{q}")
                   for q in range(NH // 2)]

            # DVE work units: conv-stack convs, head convs, head outputs.
            # gp=True routes the tap chain to the GpSimd engine (fused stt);
            # gp=False uses the DVE tensor_scalar(4x)+tensor_tensor(2x) form.
            def head_unit(i, j, gp):
                d = HEAD_DILS[i][j]
                wh = HEAD_WIN[i]
                nj = min(wh, OWN + 3 * sum(HEAD_DILS[i][j + 1:]))
                oj = wh - nj
                acc = haccp.tile([128, CTX], BF16, tag="hacc")
                nc.vector.tensor_scalar_mul(acc[:, oj:wh], xg[i][:, oj:wh],
                                               t_hww[:, i, j, 3:4])
                for k in (2, 1, 0):
                    delta = (3 - k) * d
                    a = max(oj, delta)
                    if a >= wh:
                        continue
                    nc.vector.scalar_tensor_tensor(
                        acc[:, a:wh], xg[i][:, a - delta:wh - delta],
                        t_hww[:, i, j, k:k + 1], acc[:, a:wh],
                        OP.mult, OP.add)
                nc.gpsimd.tensor_add(xg[i][:, oj:wh], xg[i][:, oj:wh],
                                       acc[:, oj:wh])

            def headout_unit(i):
                off = HEAD_WIN[i] - OWN
                cp = cpfp.tile([128, OWN], BF16, tag="cpf")
                dma(cp[:], cprof_d[i])
                nc.gpsimd.tensor_add(xg[i][:, off:], xg[i][:, off:], cp[:])
                for c in range(2):
                    cs = slice(512 * c, 512 * (c + 1))
                    pb = psp.tile([128, 512], F32, tag="ps")
                    nc.tensor.matmul(pb[:], t_oneh[:, 128 * i:128 * (i + 1)],
                                     hw_sig[:, cs], start=True, stop=True)
                    hb = sgp.tile([128, 512], BF16, tag="sg")
                    nc.scalar.activation(hb[:], pb[:], AF.Copy)
                    cso = slice(off + 512 * c, off + 512 * (c + 1))
                    nc.gpsimd.tensor_mul(xg[i][:, cso], xg[i][:, cso], hb[:])
                    # fp8 copy for the mix-gate matmul
                    nc.scalar.activation(xg8[i // 2][:, i % 2, cs],
                                         xg[i][:, cso], AF.Copy)

            head_q = [(i, j) for i in range(NH) for j in range(4)]
            head_i = [0]

            tog = [0]

            def emit_conv_unit():
                tog[0] += 1
                use_head = (tog[0] % 2 == 0) and head_i[0] < len(head_q)
                if not use_head and stack_i[0] >= len(stack_q):
                    use_head = head_i[0] < len(head_q)
                if use_head:
                    i, j = head_q[head_i[0]]
                    head_i[0] += 1
                    if j == 3:
                        headout_unit(i)
                    else:
                        stk_ctr[0] += 1
                        head_unit(i, j, gp=(stk_ctr[0] % 2 == 0))
                    return True
                return emit_stack()

            # =========== Phase F: GLU FFN (own tokens), hff in SBUF =========
            ffnES = ExitStack()
            hffp = ffnES.enter_context(tc.tile_pool(name="hff", bufs=NKI - N_OUT8))
            hff8p = ffnES.enter_context(tc.tile_pool(name="hff8", bufs=N_OUT8 // 2))
            wfip = ffnES.enter_context(tc.tile_pool(name="wfi", bufs=2))
            wfi8p = ffnES.enter_context(tc.tile_pool(name="wfi8", bufs=2))
            wfo8pp = ffnES.enter_context(tc.tile_pool(name="wfo8p", bufs=2))
            wfopp = ffnES.enter_context(tc.tile_pool(name="wfop", bufs=2))
            headES = ExitStack()
            haccp = headES.enter_context(tc.tile_pool(name="hacc", bufs=2))
            cpfp = headES.enter_context(tc.tile_pool(name="cpf", bufs=1))

            for c in range(2):
                cs_own = slice(512 * c, 512 * (c + 1))
                hffc = []      # bf16 tiles for inner k-tiles N_OUT8..47
                hff8c = []     # fp8 pair tiles for inner k-tiles 0..N_OUT8-1
                for q in range(N_OUT8 // 2):
                    hff8c.append(hff8p.tile([128, 2, 512], F8, tag="hff8", name=f"hff8_{c}_{q}"))
                for p in range(NKI):
                    wcs = wfip.tile([128, NKH, 128], BF16, tag="wfi")
                    dma(wcs[:], wfic_d[p])
                    wgs = wfi8p.tile([128, NKH, 128], F8, tag="wfi8")
                    dma(wgs[:], wfig8_d[p])
                    pc = psp.tile([128, 512], F32, tag="ps")
                    for kt in range(NKH):
                        nc.tensor.matmul(pc[:], wcs[:, kt, :],
                                         xn_o[kt][:, cs_own],
                                         start=(kt == 0), stop=(kt == NKH - 1))
                    pg = psp.tile([128, 512], F32, tag="ps")
                    for q in range(NKH // 2):
                        nc.tensor.matmul(pg[:], wgs[:, 2 * q:2 * q + 2, :],
                                         xn8o[q][:, :, cs_own],
                                         start=(q == 0), stop=(q == NKH // 2 - 1),
                                         perf_mode=DR)
                    sg = sgp.tile([128, 512], BF16, tag="sg")
                    nc.scalar.activation(sg[:], pg[:], AF.Tanh, scale=0.5)
                    if p < N_OUT8:
                        nc.vector.scalar_tensor_tensor(
                            hff8c[p // 2][:, p % 2, :], sg[:], 1.0, pc[:],
                            OP.add, OP.mult)
                    else:
                        hoc = hffp.tile([128, 512], BF16, tag="hff")
                        nc.vector.scalar_tensor_tensor(hoc[:], sg[:], 1.0,
                                                       pc[:], OP.add, OP.mult)
                        hffc.append(hoc)
                    emit_conv_unit()
                    if p % 4 == 0:
                        emit_conv_unit()

                # ffn_out: 2 groups of 6 output tiles, hff from SBUF
                for g in range(2):
                    pss = []
                    for _j in range(6):
                        psj = psp.tile([128, 512], F32, tag="ps")
                        pss.append(psj)
                    for q in range(N_OUT8 // 2):
                        w8 = wfo8pp.tile([128, 2, 6, 128], F8, tag="wfo8")
                        dma(w8[:], wfo8_d[g, q])
                        for j in range(6):
                            nc.tensor.matmul(pss[j][:], w8[:, :, j, :],
                                             hff8c[q][:, :, :],
                                             start=(q == 0), stop=False,
                                             perf_mode=DR,
                                             skip_group_check=True)
                    for kt in range(NKI - N_OUT8):
                        ws = wfopp.tile([128, 6, 128], BF16, tag="wfop")
                        nc.gpsimd.dma_start(ws[:], wfo16_d[g, kt])
                        for j in range(6):
                            nc.tensor.matmul(pss[j][:], ws[:, j, :],
                                             hffc[kt][:],
                                             start=False,
                                             stop=(kt == NKI - N_OUT8 - 1),
                                             skip_group_check=True)
                    for j in range(6):
                        pfb = sgp.tile([128, 512], BF16, tag="pcb")
                        nc.scalar.activation(pfb[:], pss[j][:], AF.Copy)
                        tf = ftp.tile([128, 512], BF16, tag="ftmp")
                        nc.gpsimd.tensor_mul(tf[:], pfb[:], gfb[:, cs_own])
                        dma(po_f[g * 6 + j, c], tf[:])
            while emit_conv_unit() or emit_stack():
                pass
            headES.close()
            ffnES.close()
            cgelp.release()
            caccp.release()

            # =========== conv-stack projection ===========
            projES = ExitStack()
            wsl = projES.enter_context(tc.tile_pool(name="wsl", bufs=2))
            gcbp = projES.enter_context(tc.tile_pool(name="gcb", bufs=1))
            gcb = pg_bcast(gcbp, 0, "gcb")
            for t in range(NKH):
                wp = wsl.tile([128, NKH, 128], BF16, tag="wsl")
                dma(wp[:], wproj_d[t])
                for c in range(2):
                    ws_ = slice(WPAD + 512 * c, WPAD + 512 * (c + 1))
                    pp = psp.tile([128, 512], F32, tag="ps")
                    for kt in range(NKH):
                        nc.tensor.matmul(pp[:], wp[:, kt, :], hs[kt][:, ws_],
                                         start=(kt == 0), stop=(kt == NKH - 1))
                    tb = sgp.tile([128, 512], BF16, tag="sg")
                    nc.scalar.activation(tb[:], pp[:], AF.Identity,
                                         bias=t_projb[:, t:t + 1])
                    tf = ftp.tile([128, 512], BF16, tag="ftmp")
                    nc.gpsimd.tensor_mul(tf[:], tb[:],
                                         gcb[:, 512 * c:512 * (c + 1)])
                    dma(po_c[t, c], tf[:])
            projES.close()
            hstp.release()

            # =========== Phase M: mix gate + mixing (fp8 DoubleRow) =========
            # sigmoid(z) = 0.5*(1+tanh(z/2)): sgm_t = tanh(0.5*pm + mgb_half),
            # xg2 = (1+sgm_t)*xg  [the 0.5 is folded into wmix on the host]
            sgmES = ExitStack()
            sgmp = sgmES.enter_context(tc.tile_pool(name="sgm", bufs=NH))
            gsbp = sgmES.enter_context(tc.tile_pool(name="gsb", bufs=1))
            gsb = pg_bcast(gsbp, 1, "gsb")
            xg82p_pool = sgmES.enter_context(tc.tile_pool(name="xg82", bufs=NH // 2))
            sgm = []
            for t in range(NKH):
                wm = w8sl.tile([128, NKH, 128], F8, tag="w8sl")
                dma(wm[:], wmg8_d[t])
                st = sgmp.tile([128, OWN], BF16, tag="sgm")
                for c in range(2):
                    pm = psp.tile([128, 512], F32, tag="ps")
                    for q in range(NKH // 2):
                        nc.tensor.matmul(pm[:], wm[:, 2 * q:2 * q + 2, :],
                                         xg8[q][:, :, 512 * c:512 * (c + 1)],
                                         start=(q == 0), stop=(q == NKH // 2 - 1),
                                         perf_mode=DR)
                    nc.scalar.activation(st[:, 512 * c:512 * (c + 1)], pm[:],
                                         AF.Tanh, bias=t_mgb[:, t:t + 1],
                                         scale=0.5)
                sgm.append(st)
            xg82 = [xg82p_pool.tile([128, 2, OWN], F8, tag="xg82", name=f"xg82_{q}")
                    for q in range(NH // 2)]
            for t in range(NKH):
                ot_ = HEAD_WIN[t] - OWN
                nc.vector.scalar_tensor_tensor(
                    xg82[t // 2][:, t % 2, :], sgm[t][:], 1.0,
                    xg[t][:, ot_:], OP.add, OP.mult)
            for t in range(NKH):
                wx = w8sl.tile([128, NKH, 128], F8, tag="w8sl")
                dma(wx[:], wmix8_d[t])
                for c in range(2):
                    pm = psp.tile([128, 512], F32, tag="ps")
                    for q in range(NKH // 2):
                        nc.tensor.matmul(pm[:], wx[:, 2 * q:2 * q + 2, :],
                                         xg82[q][:, :, 512 * c:512 * (c + 1)],
                                         start=(q == 0), stop=(q == NKH // 2 - 1),
                                         perf_mode=DR)
                    tb = sgp.tile([128, 512], BF16, tag="sg")
                    nc.scalar.activation(tb[:], pm[:], AF.Identity,
                                         bias=t_mixb[:, t:t + 1])
                    tf = ftp.tile([128, 512], BF16, tag="ftmp")
                    nc.gpsimd.tensor_mul(tf[:], tb[:],
                                         gsb[:, 512 * c:512 * (c + 1)])
                    dma(po_s[t, c], tf[:])
            sgmES.close()
            xg8p_pool.release()
            xgES.close()

    nc.finalize()
    _fix_sync_capacity(nc, dummy[:])
    return nc


# ---------------------------------------------------------------------------
# host side
# ---------------------------------------------------------------------------
def _wslab(Wt, nk, no):
    """[IN, OUT] weight (already transposed to in-major) -> [no, 128, nk, 128]
    slab layout: slab[ot][p, kt, m] = Wt[kt*128+p, ot*128+m]."""
    return np.ascontiguousarray(
        Wt.reshape(nk, 128, no, 128).transpose(2, 1, 0, 3))


def _head_bias_profile(head_ws, head_bs):
    """Data-independent bias part of each head's (linear) conv chain over the
    global sequence, with exact causal zero padding."""
    C = np.zeros((NH, HD, S), np.float32)
    for i in range(NH):
        v = np.zeros((HD, S), np.float32)
        for j, d in enumerate(HEAD_DILS[i]):
            conv = np.zeros_like(v)
            for k in range(KK):
                delta = (3 - k) * d
                if delta == 0:
                    conv += head_ws[i, j, :, 0, k][:, None] * v
                elif delta < S:
                    conv[:, delta:] += head_ws[i, j, :, 0, k][:, None] * v[:, :-delta]
            v = v + conv + head_bs[i, j][:, None]
        C[i] = v
    return C


_NC_CACHE = {}


def kernel(**inputs):
    x = np.asarray(inputs["x"], np.float32)
    nw = np.asarray(inputs["norm_w"], np.float32)
    conv_ws = np.asarray(inputs["conv_ws"], np.float32)
    conv_bs = np.asarray(inputs["conv_bs"], np.float32)
    conv_proj_w = np.asarray(inputs["conv_proj_w"], np.float32)
    conv_proj_b = np.asarray(inputs["conv_proj_b"], np.float32)
    gate_w = np.asarray(inputs["gate_w"], np.float32)
    router_w = np.asarray(inputs["router_w"], np.float32)
    router_b = np.asarray(inputs["router_b"], np.float32)
    head_ws = np.asarray(inputs["head_ws"], np.float32)
    head_bs = np.asarray(inputs["head_bs"], np.float32)
    mix_gate_w = np.asarray(inputs["mix_gate_w"], np.float32)
    mix_gate_b = np.asarray(inputs["mix_gate_b"], np.float32)
    mixing_w = np.asarray(inputs["mixing_w"], np.float32)
    mixing_b = np.asarray(inputs["mixing_b"], np.float32)
    ffn_in_w = np.asarray(inputs["ffn_in_w"], np.float32)
    ffn_out_w = np.asarray(inputs["ffn_out_w"], np.float32)
    pg_w = np.asarray(inputs["pg_w"], np.float32)
    pg_b = np.asarray(inputs["pg_b"], np.float32)

    wfi_slab = _wslab((ffn_in_w * nw[2][None, :]).T, NKH, 96)  # [96,128,12,128]
    wfo_t = ffn_out_w.T.reshape(NKI, 128, 2, 6, 128)           # kt-major
    # fp8 half: inner k-tiles 0..N_OUT8-1 as pairs [2, P, 128, 2, 6, 128]
    wfo8 = wfo_t[:N_OUT8].reshape(N_OUT8 // 2, 2, 128, 2, 6, 128) \
        .transpose(3, 0, 2, 1, 4, 5)
    wfo16 = wfo_t[N_OUT8:].transpose(2, 0, 1, 3, 4)

    shared = {
        "cw": np.ascontiguousarray(
            conv_ws[:, :, 0, :].reshape(6, NKH, 128, KK).transpose(2, 1, 0, 3)),
        "cb": np.ascontiguousarray(
            conv_bs.reshape(6, NKH, 128).transpose(2, 1, 0)),
        "nw1": np.ascontiguousarray(nw[0].reshape(NKH, 128).T),
        "hww": np.ascontiguousarray(
            head_ws[:, :, :, 0, :].transpose(2, 0, 1, 3)),
        "wg8": _wslab((np.concatenate([gate_w[:H] * 0.5, gate_w[H:]])
               * nw[1][None, :]).T, NKH, 24).astype(F8NP),
        "wr": np.ascontiguousarray(
            (router_w * nw[1][None, :]).T.reshape(NKH, 128, NH)
            .transpose(1, 0, 2)).astype(BF),
        "rb": router_b[:, None].astype(np.float32),
        "wpg": np.ascontiguousarray(
            (pg_w * nw).T.reshape(NKH, 128, 3).transpose(1, 0, 2)).astype(BF),
        "pgb": pg_b[:, None].astype(np.float32),
        "wproj": _wslab(conv_proj_w.T, NKH, NKH).astype(BF),
        "projb": np.ascontiguousarray(conv_proj_b.reshape(NKH, 128).T),
        "wmg8": _wslab(mix_gate_w.T, NKH, NKH).astype(F8NP),
        # tanh-form sigmoid: bias pre-halved; 0.5 gate factor folded into wmix
        "mgb": np.ascontiguousarray(mix_gate_b.reshape(NKH, 128).T) * 0.5,
        "wmix8": _wslab(mixing_w.T * 0.5, NKH, NKH).astype(F8NP),
        "mixb": np.ascontiguousarray(mixing_b.reshape(NKH, 128).T),
        "wfic": np.ascontiguousarray(wfi_slab[:NKI] * 0.5).astype(BF),
        "wfig8": np.ascontiguousarray(wfi_slab[NKI:]).astype(F8NP),
        "wfo8": np.ascontiguousarray(wfo8).astype(F8NP),
        "wfo16": np.ascontiguousarray(wfo16).astype(BF),
    }
    oneh = np.zeros((NH, NH * 128), np.float32)
    for i in range(NH):
        oneh[i, 128 * i:128 * (i + 1)] = 1.0
    shared["oneh"] = oneh.astype(BF)

    cprof = _head_bias_profile(head_ws, head_bs)  # [NH, HD, S]
    cprof_h = [
        np.ascontiguousarray(cprof[:, :, h * OWN:(h + 1) * OWN]).astype(BF)
        for h in range(2)
    ]
    mask_h = []
    m0 = np.zeros((128, W), np.float32)
    m0[:, WPAD:] = 1.0
    mask_h.append(m0.astype(BF))
    mask_h.append(np.ones((128, W), BF))

    in_maps = []
    for core in range(N_CORES):
        b, h = core // 2, core % 2
        if h == 0:
            ctx = np.concatenate(
                [np.zeros((OWN, H), np.float32), x[b, :OWN]], axis=0)
        else:
            ctx = x[b]
        xc = np.ascontiguousarray(ctx.T.reshape(NKH, 128, CTX)).astype(BF)
        m = dict(shared)
        m["xc"] = xc
        m["cprof"] = cprof_h[h]
        m["mask"] = mask_h[h]
        in_maps.append(m)

    key = "nc"
    if key not in _NC_CACHE:
        _NC_CACHE[key] = _build()
    nc = _NC_CACHE[key]

    import os
    trace = bool(os.environ.get("BASS_KERNEL_TRACE"))
    r = run_bass_kernel_spmd(nc, in_maps, list(range(N_CORES)), trace=trace)
    global LAST_EXEC_NS
    LAST_EXEC_NS = r.exec_time_ns
    res = r.results

    out = np.empty((B, S, H), np.float32)
    for core in range(N_CORES):
        b, h = core // 2, core % 2
        total = np.zeros((H, OWN), np.float32)
        for name in ("po_c", "po_s", "po_f"):
            arr = np.asarray(res[core][name]).astype(np.float32)
            total += arr.transpose(0, 2, 1, 3).reshape(H, OWN)
        rows = slice(h * OWN, (h + 1) * OWN)
        out[b, rows, :] = x[b, rows, :] + total.T
    return out
